# revision 24
# baseline (speedup 1.0000x reference)
"""DGCNN (4x SAGEConv + SortPool + Conv1d + MLP) Trainium2 Bass kernel, v2.

Sharding: data-parallel over the B=512 graphs -> 64 graphs per core on 8 cores.
Edges never cross graphs, so each core's message passing is local; aggregation
is a block-diagonal dense matmul (2 graphs of 64 nodes per 128-partition tile).

Numerics: the SortPool ordering must match the fp32 reference almost exactly
(adjacent sort-key gaps go down to ~6e-7), so the SAGE chain uses an
fp32-faithful fp16 two-term split everywhere instead of native fp32 matmuls:
  h = h1 + h2 (fp16 pair, residual ~2^-24), w = w1 + w2
  h @ w ~= h1@w1 + h1@w2 + h2@w1   (3 passes at 1 cyc/col vs fp32's 4)
The aggregation uses the raw integer edge-multiplicity matrix A (exact in
fp16) and applies 1/deg afterwards on the vector engine, so
  agg = (A^T (h1+h2)) * inv_deg    (2 passes, exact products, fp32 PSUM)
Verified on-HW: fp16 split matmul reaches 2.2e-7 rel err; PE handles fp16
subnormals exactly. Everything after the sort (selection, conv1d, MLP) only
needs magnitude accuracy and runs in plain fp16 (~2e-4), final MLP in fp32.

SortPool is exact (stable argsort semantics incl. ties) via a rank
computation on keys perturbed by -i*1e-11 (resolves exact ties by index).
"""

import numpy as np

import concourse.bass as bass
import concourse.bacc as bacc
import concourse.mybir as mybir
import concourse.tile as tile
from concourse.bass_utils import run_bass_kernel_spmd

B, P, K, KS = 512, 64, 30, 4
N, E, F, H = B * P, 524288, 128, 256
L_OUT = K - KS + 1          # 27
N_CLASSES = 10
N_CORES = 8
GPC = B // N_CORES          # 64 graphs / core
NPC = GPC * P               # 4096 nodes / core
PAIRS = GPC // 2            # 32 pair-tiles (2 graphs of 64 nodes = 128 partitions)
NCHUNK = 512                # free-dim chunk for weight matmuls
F32 = mybir.dt.float32
FP16 = mybir.dt.float16
EPS_TIE = 1e-11

NLAYERS = 4
GCHUNK = 16                 # graphs per conv psum tile (16*28 = 448 <= 512)
L28 = L_OUT + 1             # conv free dim padded even
TKPAD = GPC * K + 8         # topkT free size incl. zeroed overrun pad
S1 = 2 * L_OUT              # 54 lin1 contraction steps of 128


def _split16(a):
    """fp32 -> (hi, lo) fp16 pair with ~2^-24 residual."""
    a = np.asarray(a, np.float32)
    hi = a.astype(np.float16)
    lo = (a - hi.astype(np.float32)).astype(np.float16)
    return hi, lo


# ---------------------------------------------------------------- host prep

def _pmaj(a):
    """[K*128, ...] -> partition-major [128, K, ...] contiguous."""
    a = np.asarray(a)
    k = a.shape[0] // 128
    return np.ascontiguousarray(
        a.reshape((k, 128) + a.shape[1:]).transpose((1, 0) + tuple(range(2, a.ndim + 1))))


def _prep_shared(inp):
    """Host-side weight/constant reshaping (identical for every core).

    Everything is pre-rearranged to the on-SBUF partition-major layout so
    every DMA is a plain per-partition contiguous copy.
    """
    sh = {}
    for li in range(4):
        wl1, wl2 = _split16(inp[f"sage{li}_wl"])
        wr1, wr2 = _split16(inp[f"sage{li}_wr"])
        sh[f"wl1_{li}"], sh[f"wl2_{li}"] = _pmaj(wl1), _pmaj(wl2)
        sh[f"wr1_{li}"], sh[f"wr2_{li}"] = _pmaj(wr1), _pmaj(wr2)
        sh[f"b{li}"] = np.ascontiguousarray(
            np.asarray(inp[f"sage{li}_b"], np.float32).reshape(2, 128).T)
    w = np.asarray(inp["conv1d_w"], np.float32)            # [O=256, I=256, KS]
    w2 = np.empty((2 * KS, 128, H), np.float16)
    for k in range(KS):
        wt = w[:, :, k].T.astype(np.float16)               # [I, O]
        for ih in range(2):
            w2[k * 2 + ih] = wt[ih * 128:(ih + 1) * 128]
    sh["w2"] = np.ascontiguousarray(w2.transpose(1, 0, 2))       # [128, 2KS, H]
    sh["cb"] = np.ascontiguousarray(
        np.asarray(inp["conv1d_b"], np.float32).reshape(2, 128).T)
    w1 = np.asarray(inp["lin1_w"], np.float32)             # [6912, 256]
    sh["w1"] = np.ascontiguousarray(
        w1.reshape(2, 128, L_OUT, H).transpose(0, 2, 1, 3).reshape(S1, 128, H)
        .transpose(1, 0, 2)).astype(np.float16)                  # [128, S1, H]
    sh["lb1"] = np.ascontiguousarray(
        np.broadcast_to(np.asarray(inp["lin1_b"], np.float32), (GPC, H)))
    sh["w4"] = _pmaj(np.asarray(inp["lin2_w"], np.float32))      # [128, 2, 128]
    sh["b2q"] = np.ascontiguousarray(inp["lin2_b"], np.float32).reshape(128, 1)
    sh["w5"] = np.ascontiguousarray(inp["out_w"], np.float32)    # [128, 10]
    sh["b3q"] = np.asarray(inp["out_b"], np.float32).reshape(N_CLASSES, 1).copy()
    sh["iota60"] = np.ascontiguousarray(
        np.broadcast_to(np.arange(2 * K, dtype=np.float32), (128, 2 * K)))
    off30 = np.zeros((128, 1), np.float32)
    off30[64:] = float(K)
    sh["off30"] = off30
    sh["epsrow"] = np.ascontiguousarray(
        np.broadcast_to(np.arange(P, dtype=np.float32) * np.float32(EPS_TIE), (P, P))).astype(np.float32)
    sh["id128"] = np.eye(128, dtype=np.float32)
    sh["id16"] = np.eye(128, dtype=np.float16)
    return sh


def _prep_cores(inp):
    """Per-core shards: split node features and integer blockdiag adjacency."""
    x = np.nan_to_num(np.asarray(inp["x"], np.float32))
    ei = np.asarray(inp["edge_index"])
    src = ei[0].astype(np.int64)
    dst = ei[1].astype(np.int64)
    deg = np.bincount(dst, minlength=N).astype(np.float32)
    inv_deg = (1.0 / np.maximum(deg, 1.0)).astype(np.float32)
    g = src // P
    flat = g * (P * P) + (src % P) * P + (dst % P)
    A = np.bincount(flat, minlength=B * P * P).astype(np.float16).reshape(B, P, P)

    cores = []
    for c in range(N_CORES):
        xc = x[c * NPC:(c + 1) * NPC]                            # [4096, 128]
        x1, x2 = _split16(xc)
        x1t = np.ascontiguousarray(x1.T)                         # fp16(x).T
        x2t = np.ascontiguousarray(x2.T)
        abd = np.zeros((PAIRS, 128, 128), np.float16)
        for t in range(PAIRS):
            abd[t, :P, :P] = A[c * GPC + 2 * t]
            abd[t, P:, P:] = A[c * GPC + 2 * t + 1]
        cores.append({
            "x1": _pmaj(x1), "x2": _pmaj(x2),                    # [128, PAIRS, F]
            "x1t": x1t, "x2t": x2t,
            "abd": np.ascontiguousarray(abd.transpose(1, 0, 2)),  # [128, PAIRS, 128]
            "invbc": np.ascontiguousarray(np.broadcast_to(
                inv_deg[c * NPC:(c + 1) * NPC], (128, NPC))),
        })
    return cores


# ---------------------------------------------------------------- device kernel

def _build(nc):
    dt = nc.dram_tensor
    d = {}
    d["d_x1"] = dt("x1", [128, PAIRS, F], FP16, kind="ExternalInput")
    d["d_x2"] = dt("x2", [128, PAIRS, F], FP16, kind="ExternalInput")
    d["d_x1t"] = dt("x1t", [F, NPC], FP16, kind="ExternalInput")
    d["d_x2t"] = dt("x2t", [F, NPC], FP16, kind="ExternalInput")
    d["d_abd"] = dt("abd", [128, PAIRS, 128], FP16, kind="ExternalInput")
    d["d_invbc"] = dt("invbc", [128, NPC], F32, kind="ExternalInput")
    for li in range(4):
        ki = 1 if li == 0 else 2
        for nm in ("wl1", "wl2", "wr1", "wr2"):
            d[f"d_{nm}_{li}"] = dt(f"{nm}_{li}", [128, ki, H], FP16, kind="ExternalInput")
        d[f"d_b{li}"] = dt(f"b{li}", [128, 2], F32, kind="ExternalInput")
    d["d_w2"] = dt("w2", [128, 2 * KS, H], FP16, kind="ExternalInput")
    d["d_cb"] = dt("cb", [128, 2], F32, kind="ExternalInput")
    d["d_w1"] = dt("w1", [128, S1, H], FP16, kind="ExternalInput")
    d["d_lb1"] = dt("lb1", [GPC, H], F32, kind="ExternalInput")
    d["d_w4"] = dt("w4", [128, 2, 128], F32, kind="ExternalInput")
    d["d_b2q"] = dt("b2q", [128, 1], F32, kind="ExternalInput")
    d["d_w5"] = dt("w5", [128, N_CLASSES], F32, kind="ExternalInput")
    d["d_b3q"] = dt("b3q", [N_CLASSES, 1], F32, kind="ExternalInput")
    d["d_iota60"] = dt("iota60", [128, 2 * K], F32, kind="ExternalInput")
    d["d_off30"] = dt("off30", [128, 1], F32, kind="ExternalInput")
    d["d_epsrow"] = dt("epsrow", [P, P], F32, kind="ExternalInput")
    d["d_id128"] = dt("id128", [128, 128], F32, kind="ExternalInput")
    d["d_id16"] = dt("id16", [128, 128], FP16, kind="ExternalInput")
    d["d_out"] = dt("out", [GPC, N_CLASSES], F32, kind="ExternalOutput")

    with tile.TileContext(nc) as tc:
        _emit(tc, nc, d)
    nc.compile()
    return nc


def _ap(base, extra_offset, free_dims):
    """Custom AP view: replace partition+free dims relative to a tile AP."""
    return bass.AP(base.tensor, base.offset + extra_offset,
                   [base.ap[0]] + list(free_dims))


def _bap(base, extra_offset, dims):
    """AP with explicit partition dim (e.g. broadcast stride-0)."""
    return bass.AP(base.tensor, base.offset + extra_offset, list(dims))


def _emit(tc, nc, d):
    from contextlib import ExitStack
    ctx = ExitStack()
    with ctx:
        persist = ctx.enter_context(tc.tile_pool(name="persist", bufs=1))
        act_pool = ctx.enter_context(tc.tile_pool(name="acts", bufs=1))

        _deferred = []

        def load(name, shape, dram=None, dtype=F32):
            t = persist.tile(shape, dtype, tag=name)
            _deferred.append((t, (dram if dram is not None else d[f"d_{name}"]).ap()))
            return t

        wl1, wl2, wr1, wr2, bias = [], [], [], [], []
        for li in range(4):
            ki = 1 if li == 0 else 2
            for lst, nm in ((wl1, "wl1"), (wl2, "wl2"), (wr1, "wr1"), (wr2, "wr2")):
                lst.append(load(f"{nm}_{li}", [128, ki, H],
                                dram=d[f"d_{nm}_{li}"], dtype=FP16))
            bias.append(load(f"b{li}", [128, 2], dram=d[f"d_b{li}"]))
        id16 = load("id16", [128, 128], dtype=FP16)
        iota60 = load("iota60", [128, 2 * K])
        off30 = load("off30", [128, 1])
        epsrow = load("epsrow", [P, P])
        id128 = load("id128", [128, 128])
        w2 = load("w2", [128, 2 * KS, H], dtype=FP16)
        cb = load("cb", [128, 2])
        b1 = load("lb1", [GPC, H])
        w4 = load("w4", [128, 2, 128])
        b2q = load("b2q", [128, 1])
        w5 = load("w5", [128, N_CLASSES])
        b3q = load("b3q", [N_CLASSES, 1])
        inv_bc = persist.tile([128, NPC], F32, tag="invbc")
        w1sb = load("w1", [128, S1, H], dtype=FP16)

        # ---- activations that outlive the SAGE phase
        h1_sb = act_pool.tile([128, PAIRS, H], FP16, tag="h1")
        h2_sb = act_pool.tile([128, PAIRS, H], FP16, tag="h2")
        rt = act_pool.tile([P, P], F32, tag="rt")       # sort ranks, transposed

        with tc.tile_pool(name="sage", bufs=1) as sg:
            hts = [(sg.tile([128, 2, NPC], FP16, tag="hT1a", name="hT1a"),
                    sg.tile([128, 2, NPC], FP16, tag="hT2a", name="hT2a")),
                   (sg.tile([128, 2, NPC], FP16, tag="hT1b", name="hT1b"),
                    sg.tile([128, 2, NPC], FP16, tag="hT2b", name="hT2b"))]
            aggT1 = sg.tile([128, 2, NPC], FP16, tag="aggT1")
            aggT2 = sg.tile([128, 2, NPC], FP16, tag="aggT2")

            # ---- input DMAs, priority-ordered across 5 engine queues.
            # x pair lands straight in h1/h2_sb fh=0; xT pair in hts[0] slot 0
            # (they act as the "layer -1" activations).
            abd_sb = persist.tile([128, PAIRS, 128], FP16, tag="abd")
            first = [
                (_ap(h1_sb[:, :], 0, [[H, PAIRS], [1, F]]), d["d_x1"].ap()),
                (abd_sb[...], d["d_abd"].ap()),
                (_ap(h2_sb[:, :], 0, [[H, PAIRS], [1, F]]), d["d_x2"].ap()),
                (inv_bc[...], d["d_invbc"].ap()),
                (hts[0][0][:, 0, :], d["d_x1t"].ap()),
                (hts[0][1][:, 0, :], d["d_x2t"].ap()),
            ]
            queues = [nc.sync, nc.scalar, nc.gpsimd]
            for _i, (_dst, _src) in enumerate(first + _deferred):
                queues[_i % len(queues)].dma_start(
                    _dst if isinstance(_dst, bass.AP) else _dst[...], _src)
            _deferred.clear()

            with tc.tile_pool(name="ps_agg", bufs=2, space="PSUM") as psa, \
                 tc.tile_pool(name="ps_w", bufs=2, space="PSUM") as psw, \
                 tc.tile_pool(name="ps_tr", bufs=2, space="PSUM") as pst, \
                 tc.tile_pool(name="scr", bufs=2) as scr:
                for li in range(NLAYERS):
                    ki = 1 if li == 0 else 2
                    hT1v, hT2v = hts[li % 2]
                    hT1o, hT2o = hts[(li + 1) % 2]

                    # ---- agg: aggT[f, d] = (sum_s h[s,f] A[s,d]) * invdeg[d]
                    for g4 in range(PAIRS // 4):
                        for fh in range(ki):
                            ps = psa.tile([128, 4, 128], F32, tag="psa")
                            for j in range(4):
                                t = g4 * 4 + j
                                sl = slice(fh * 128, (fh + 1) * 128)
                                lh1, lh2 = h1_sb[:, t, sl], h2_sb[:, t, sl]
                                rhs = abd_sb[:, t, :]
                                nc.tensor.matmul(ps[:, j, :], lhsT=lh1, rhs=rhs,
                                                 start=True, stop=False)
                                nc.tensor.matmul(ps[:, j, :], lhsT=lh2, rhs=rhs,
                                                 start=False, stop=True)
                            tmp = scr.tile([128, 512], F32, tag="tmp")
                            nsl = slice(g4 * 512, (g4 + 1) * 512)
                            nc.vector.tensor_tensor(
                                tmp[...], _ap(ps[:, :], 0, [[1, 512]]),
                                inv_bc[:, nsl], op=mybir.AluOpType.mult)
                            nc.gpsimd.tensor_copy(aggT1[:, fh, nsl], tmp[...])
                            nc.gpsimd.tensor_tensor(
                                aggT2[:, fh, nsl], tmp[...], aggT1[:, fh, nsl],
                                op=mybir.AluOpType.subtract)

                    # ---- weights: hT_next = relu(wl^T agg + wr^T h + b)
                    oh_order = (1, 0) if li == NLAYERS - 1 else (0, 1)
                    for oh in oh_order:
                        osl = slice(oh * 128, (oh + 1) * 128)
                        for ncki in range(NPC // NCHUNK):
                            sl = slice(ncki * NCHUNK, (ncki + 1) * NCHUNK)
                            ps = psw.tile([128, NCHUNK], F32, tag="psw")
                            prods = []
                            for m1, m2, r1, r2 in (
                                    (wl1[li], wl2[li], aggT1, aggT2),
                                    (wr1[li], wr2[li], hT1v, hT2v)):
                                for fh in range(ki):
                                    prods += [(m1, fh, r1), (m1, fh, r2),
                                              (m2, fh, r1)]
                            for i, (wm, fh, rt_) in enumerate(prods):
                                nc.tensor.matmul(
                                    ps[...], lhsT=wm[:, fh, osl],
                                    rhs=rt_[:, fh, sl],
                                    start=(i == 0), stop=(i == len(prods) - 1))
                            nc.scalar.activation(
                                hT1o[:, oh, sl], ps[...],
                                mybir.ActivationFunctionType.Relu,
                                bias=bias[li][:, oh:oh + 1])
                            if li < NLAYERS - 1 or oh == 1:
                                # (L3 needs hT2 only for the sort-key channel
                                # 255, i.e. the oh=1 half, row 127)
                                nc.vector.scalar_tensor_tensor(
                                    hT2o[:, oh, sl], ps[...], 0.0,
                                    hT1o[:, oh, sl],
                                    op0=mybir.AluOpType.max,
                                    op1=mybir.AluOpType.subtract)

                    # ---- transpose hT_next -> node-major h (fp16, batched 2 pairs)
                    parts = ((hT1o, h1_sb), (hT2o, h2_sb)) if li < NLAYERS - 1 \
                        else ((hT1o, h1_sb),)
                    for pi, (src_t, dst_t) in enumerate(parts):
                        for t0 in range(0, PAIRS, 2):
                            ps = pst.tile([128, 4, 128], FP16, tag="pst")
                            for dt_ in range(2):
                                for oh in range(2):
                                    nc.tensor.transpose(
                                        ps[:, dt_ * 2 + oh, :],
                                        src_t[:, oh, (t0 + dt_) * 128:(t0 + dt_ + 1) * 128],
                                        id16[...])
                            if (t0 // 2 + pi) % 2 == 0:
                                nc.scalar.activation(
                                    _ap(dst_t[:, :], t0 * H, [[1, 2 * H]]),
                                    _ap(ps[:, :], 0, [[1, 512]]),
                                    mybir.ActivationFunctionType.Copy)
                            else:
                                nc.vector.tensor_copy(
                                    _ap(dst_t[:, :], t0 * H, [[1, 2 * H]]),
                                    _ap(ps[:, :], 0, [[1, 512]]))

            # ---- sort keys: channel 255 = row 127 of (hT1+hT2)(L3)[:,1,:]
            with tc.tile_pool(name="sort", bufs=1) as ss:
                km1 = ss.tile([P, P], FP16, tag="km1")
                nc.sync.dma_start(km1[...], hts[NLAYERS % 2][0][127:128, 1, :])
                km2 = ss.tile([P, P], FP16, tag="km2")
                nc.scalar.dma_start(km2[...], hts[NLAYERS % 2][1][127:128, 1, :])
                kmf = ss.tile([P, P], F32, tag="kmf")
                nc.vector.tensor_add(kmf[...], km1[...], km2[...])
                kmp = ss.tile([P, P], F32, tag="kmp")
                nc.vector.tensor_sub(kmp[...], kmf[...], epsrow[...])
                cbt = ss.tile([P, P * P], mybir.dt.uint8, tag="cbt")
                kb = kmp[:, :]
                in0 = _ap(kb, 0, [[0, P], kb.ap[1]])       # [g, i(bc), j] k(g, j)
                in1 = _ap(kb, 0, [kb.ap[1], [0, P]])       # [g, i, j(bc)] k(g, i)
                nc.vector.tensor_tensor(
                    _ap(cbt[:, :], 0, [[P, P], [1, P]]), in0, in1,
                    op=mybir.AluOpType.is_gt)
                rk = ss.tile([P, P], F32, tag="rk")
                nc.vector.tensor_reduce(
                    rk[...], _ap(cbt[:, :], 0, [[P, P], [1, P]]),
                    axis=mybir.AxisListType.X, op=mybir.AluOpType.add)
                with tc.tile_pool(name="ps_sort", bufs=1, space="PSUM") as pss:
                    pr = pss.tile([P, P], F32, tag="pr")
                    nc.tensor.transpose(pr[...], rk[...], id128[0:P, 0:P])
                    nc.any.tensor_copy(rt[...], pr[...])

        # ---------------- selection + conv + mlp (sage pool closed)
        with tc.tile_pool(name="tail", bufs=1) as tp, \
             tc.tile_pool(name="ps_tail", bufs=2, space="PSUM") as ptl, \
             tc.tile_pool(name="ps_fin", bufs=1, space="PSUM") as pfin:
            # rankP[p, t] = rank(node p%64 of graph 2t + p//64)
            rankp = tp.tile([128, PAIRS], F32, tag="rankp")
            rb = rt[:, :]
            nc.vector.tensor_copy(rankp[0:P, :], _ap(rb, 0, [[2, PAIRS]]))
            nc.sync.dma_start(rankp[P:128, :], _ap(rb, 1, [[2, PAIRS]]))
            ge30 = tp.tile([128, PAIRS], F32, tag="ge30")
            nc.vector.tensor_scalar(ge30[...], rankp[...], float(K), None,
                                    op0=mybir.AluOpType.is_ge)
            rank2 = tp.tile([128, PAIRS], F32, tag="rank2")
            nc.vector.scalar_tensor_tensor(rank2[...], ge30[...], 1000.0,
                                           rankp[...], op0=mybir.AluOpType.mult,
                                           op1=mybir.AluOpType.add)
            nc.vector.tensor_scalar(rank2[...], rank2[...], off30[:, 0:1], None,
                                    op0=mybir.AluOpType.add)
            # one-hot selection matrices  PT[p, t, c] = (c == rank2[p, t])
            pt_all = tp.tile([128, PAIRS, 2 * K], FP16, tag="pt")
            io = iota60[:, :]
            r2 = rank2[:, :]
            nc.vector.tensor_tensor(
                pt_all[...],
                _ap(io, 0, [[0, PAIRS], [1, 2 * K]]),
                _ap(r2, 0, [[1, PAIRS], [0, 2 * K]]),
                op=mybir.AluOpType.is_equal)

            # topkT[f, b*30+r] = sum_n h1[n, f] * PT[n, b(pair), r]  (fp16)
            topkT = tp.tile([128, 2, TKPAD], FP16, tag="topkT")
            nc.vector.memset(topkT[:, :, GPC * K:].bitcast(F32), 0.0)
            for t0 in range(0, PAIRS, 4):
                ps = ptl.tile([128, 8, 2 * K], F32, tag="pssel")
                for j in range(4):
                    for mh in range(2):
                        nc.tensor.matmul(
                            ps[:, j * 2 + mh, :],
                            lhsT=h1_sb[:, t0 + j, mh * 128:(mh + 1) * 128],
                            rhs=pt_all[:, t0 + j, :],
                            start=True, stop=True)
                for mh in range(2):
                    nc.any.tensor_copy(
                        _ap(topkT[:, mh, :], t0 * 2 * K, [[1, 4 * 2 * K]]),
                        _bap(ps[:, :], mh * 2 * K, [ps.ap[0], [4 * K, 4], [1, 2 * K]]))

            # conv1d: y[p, oh, b, l] = relu(sum_{k, ih} w2^T topkT[:, b*30+l+k] + cb)
            y_sb = tp.tile([128, 2, GPC, L28], FP16, tag="y")
            for oh in range(2):
                for bc in range(GPC // GCHUNK):
                    ps = ptl.tile([128, GCHUNK, L28], F32, tag="psconv")
                    step = 0
                    for k in range(KS):
                        for ih in range(2):
                            base = topkT[:, ih, :]
                            rhs = _ap(base, bc * GCHUNK * K + k,
                                      [[K, GCHUNK], [1, L28]])
                            nc.tensor.matmul(
                                ps[...],
                                lhsT=w2[:, k * 2 + ih, oh * 128:(oh + 1) * 128],
                                rhs=rhs,
                                start=(step == 0), stop=(step == 2 * KS - 1))
                            step += 1
                    nc.scalar.activation(
                        y_sb[:, oh, bc * GCHUNK:(bc + 1) * GCHUNK, :], ps[...],
                        mybir.ActivationFunctionType.Relu,
                        bias=cb[:, oh:oh + 1])

            # lin1 (b-major): z1T[b, o] = relu(sum_s y_s^T @ w1_s + b1)
            ps1 = pfin.tile([GPC, H], F32, tag="ps1")
            for s in range(S1):
                ot, l = divmod(s, L_OUT)
                nc.tensor.matmul(
                    ps1[...],
                    lhsT=y_sb[:, ot, :, l],
                    rhs=w1sb[:, s, :],
                    start=(s == 0), stop=(s == S1 - 1))
            z1t = tp.tile([GPC, H], F32, tag="z1t")
            nc.vector.tensor_add(z1t[...], ps1[...], b1[...])
            nc.scalar.activation(z1t[...], z1t[...],
                                 mybir.ActivationFunctionType.Relu, bias=0.0)
            z1 = tp.tile([128, 2, GPC], F32, tag="z1")
            for mh in range(2):
                psz = pfin.tile([128, GPC], F32, tag="psz")
                nc.tensor.transpose(psz[...],
                                    z1t[:, mh * 128:(mh + 1) * 128],
                                    id128[0:GPC, 0:GPC])
                nc.any.tensor_copy(z1[:, mh, :], psz[...])

            # lin2 + out
            ps2 = pfin.tile([128, GPC], F32, tag="ps2")
            for kh in range(2):
                nc.tensor.matmul(ps2[...], lhsT=w4[:, kh, :], rhs=z1[:, kh, :],
                                 start=(kh == 0), stop=(kh == 1))
            z2 = tp.tile([128, GPC], F32, tag="z2")
            nc.scalar.activation(z2[...], ps2[...],
                                 mybir.ActivationFunctionType.Relu,
                                 bias=b2q[:, 0:1])
            ps3 = pfin.tile([N_CLASSES, GPC], F32, tag="ps3")
            nc.tensor.matmul(ps3[...], lhsT=w5[...], rhs=z2[...],
                             start=True, stop=True)
            o_sb = tp.tile([N_CLASSES, GPC], F32, tag="osb")
            nc.scalar.activation(o_sb[...], ps3[...],
                                 mybir.ActivationFunctionType.Relu,
                                 bias=b3q[:, 0:1])
            nc.sync.dma_start(d["d_out"].ap().rearrange("b o -> o b"), o_sb[...])


# ---------------------------------------------------------------- entry point

_CACHED = {}


def _get_nc():
    if "nc" not in _CACHED:
        nc = bacc.Bacc("TRN2", target_bir_lowering=False, debug=False,
                       enable_asserts=True)
        _CACHED["nc"] = _build(nc)
    return _CACHED["nc"]


def make_in_maps(inputs):
    sh = _prep_shared(inputs)
    cores = _prep_cores(inputs)
    return [{**sh, **c} for c in cores]


TRACE = False


def kernel(**inputs):
    in_maps = make_in_maps(inputs)
    nc = _get_nc()
    res = run_bass_kernel_spmd(nc, in_maps, core_ids=list(range(N_CORES)),
                               trace=TRACE)
    _CACHED["last_res"] = res
    return np.concatenate([r["out"] for r in res.results], axis=0)


if __name__ == "__main__":
    import reference
    inputs = {k: np.asarray(v) for k, v in reference.setup_inputs().items()}
    out = kernel(**inputs)
    print("out", out.shape, out.dtype)


# revision 25
# speedup vs baseline: 1.5094x; 1.5094x over previous
"""DGCNN (4x SAGEConv + SortPool + Conv1d + MLP) Trainium2 Bass kernel, v2.

Sharding: data-parallel over the B=512 graphs -> 64 graphs per core on 8 cores.
Edges never cross graphs, so each core's message passing is local; aggregation
is a block-diagonal dense matmul (2 graphs of 64 nodes per 128-partition tile).

Numerics: the SortPool ordering must match the fp32 reference almost exactly
(adjacent sort-key gaps go down to ~6e-7), so the SAGE chain uses an
fp32-faithful fp16 two-term split everywhere instead of native fp32 matmuls:
  h = h1 + h2 (fp16 pair, residual ~2^-24), w = w1 + w2
  h @ w ~= h1@w1 + h1@w2 + h2@w1   (3 passes at 1 cyc/col vs fp32's 4)
The aggregation uses the raw integer edge-multiplicity matrix A (exact in
fp16) and applies 1/deg afterwards on the vector engine, so
  agg = (A^T (h1+h2)) * inv_deg    (2 passes, exact products, fp32 PSUM)
Verified on-HW: fp16 split matmul reaches 2.2e-7 rel err; PE handles fp16
subnormals exactly. Everything after the sort (selection, conv1d, MLP) only
needs magnitude accuracy and runs in plain fp16 (~2e-4), final MLP in fp32.

SortPool is exact (stable argsort semantics incl. ties) via a rank
computation on keys perturbed by -i*1e-11 (resolves exact ties by index).
"""

import numpy as np

import concourse.bass as bass
import concourse.bacc as bacc
import concourse.mybir as mybir
import concourse.tile as tile
from concourse.bass_utils import run_bass_kernel_spmd

B, P, K, KS = 512, 64, 30, 4
N, E, F, H = B * P, 524288, 128, 256
L_OUT = K - KS + 1          # 27
N_CLASSES = 10
N_CORES = 8
GPC = B // N_CORES          # 64 graphs / core
NPC = GPC * P               # 4096 nodes / core
PAIRS = GPC // 2            # 32 pair-tiles (2 graphs of 64 nodes = 128 partitions)
NCHUNK = 512                # free-dim chunk for weight matmuls
F32 = mybir.dt.float32
FP16 = mybir.dt.float16
EPS_TIE = 1e-11

NLAYERS = 4
GCHUNK = 16                 # graphs per conv psum tile (16*28 = 448 <= 512)
L28 = L_OUT + 1             # conv free dim padded even
TKPAD = GPC * K + 8         # topkT free size incl. zeroed overrun pad
S1 = 2 * L_OUT              # 54 lin1 contraction steps of 128


def _split16(a):
    """fp32 -> (hi, lo) fp16 pair with ~2^-24 residual."""
    a = np.asarray(a, np.float32)
    hi = a.astype(np.float16)
    lo = (a - hi.astype(np.float32)).astype(np.float16)
    return hi, lo


# ---------------------------------------------------------------- host prep

def _pmaj(a):
    """[K*128, ...] -> partition-major [128, K, ...] contiguous."""
    a = np.asarray(a)
    k = a.shape[0] // 128
    return np.ascontiguousarray(
        a.reshape((k, 128) + a.shape[1:]).transpose((1, 0) + tuple(range(2, a.ndim + 1))))


def _prep_shared(inp):
    """Host-side weight/constant reshaping (identical for every core).

    Everything is pre-rearranged to the on-SBUF partition-major layout so
    every DMA is a plain per-partition contiguous copy.
    """
    sh = {}
    for li in range(4):
        wl1, wl2 = _split16(inp[f"sage{li}_wl"])
        wr1, wr2 = _split16(inp[f"sage{li}_wr"])
        sh[f"wl1_{li}"], sh[f"wl2_{li}"] = _pmaj(wl1), _pmaj(wl2)
        sh[f"wr1_{li}"], sh[f"wr2_{li}"] = _pmaj(wr1), _pmaj(wr2)
        sh[f"b{li}"] = np.ascontiguousarray(
            np.asarray(inp[f"sage{li}_b"], np.float32).reshape(2, 128).T)
    w = np.asarray(inp["conv1d_w"], np.float32)            # [O=256, I=256, KS]
    w2 = np.empty((2 * KS, 128, H), np.float16)
    for k in range(KS):
        wt = w[:, :, k].T.astype(np.float16)               # [I, O]
        for ih in range(2):
            w2[k * 2 + ih] = wt[ih * 128:(ih + 1) * 128]
    sh["w2"] = np.ascontiguousarray(w2.transpose(1, 0, 2))       # [128, 2KS, H]
    sh["cb"] = np.ascontiguousarray(
        np.asarray(inp["conv1d_b"], np.float32).reshape(2, 128).T)
    w1 = np.asarray(inp["lin1_w"], np.float32)             # [6912, 256]
    sh["w1"] = np.ascontiguousarray(
        w1.reshape(2, 128, L_OUT, H).transpose(0, 2, 1, 3).reshape(S1, 128, H)
        .transpose(1, 0, 2)).astype(np.float16)                  # [128, S1, H]
    sh["lb1"] = np.ascontiguousarray(
        np.broadcast_to(np.asarray(inp["lin1_b"], np.float32), (GPC, H)))
    sh["w4"] = _pmaj(np.asarray(inp["lin2_w"], np.float32))      # [128, 2, 128]
    sh["b2q"] = np.ascontiguousarray(inp["lin2_b"], np.float32).reshape(128, 1)
    sh["w5"] = np.ascontiguousarray(inp["out_w"], np.float32)    # [128, 10]
    sh["b3q"] = np.asarray(inp["out_b"], np.float32).reshape(N_CLASSES, 1).copy()
    sh["iota60"] = np.ascontiguousarray(
        np.broadcast_to(np.arange(2 * K, dtype=np.float32), (128, 2 * K)))
    off30 = np.zeros((128, 1), np.float32)
    off30[64:] = float(K)
    sh["off30"] = off30
    sh["epsrow"] = np.ascontiguousarray(
        np.broadcast_to(np.arange(P, dtype=np.float32) * np.float32(EPS_TIE), (P, P))).astype(np.float32)
    sh["id128"] = np.eye(128, dtype=np.float32)
    sh["id16"] = np.eye(128, dtype=np.float16)
    return sh


def _prep_cores(inp):
    """Per-core shards: split node features and integer blockdiag adjacency."""
    x = np.nan_to_num(np.asarray(inp["x"], np.float32))
    ei = np.asarray(inp["edge_index"])
    src = ei[0].astype(np.int64)
    dst = ei[1].astype(np.int64)
    deg = np.bincount(dst, minlength=N).astype(np.float32)
    inv_deg = (1.0 / np.maximum(deg, 1.0)).astype(np.float32)
    g = src // P
    flat = g * (P * P) + (src % P) * P + (dst % P)
    A = np.bincount(flat, minlength=B * P * P).astype(np.float16).reshape(B, P, P)

    cores = []
    for c in range(N_CORES):
        xc = x[c * NPC:(c + 1) * NPC]                            # [4096, 128]
        x1, x2 = _split16(xc)
        x1t = np.ascontiguousarray(x1.T)                         # fp16(x).T
        x2t = np.ascontiguousarray(x2.T)
        abd = np.zeros((PAIRS, 128, 128), np.float16)
        for t in range(PAIRS):
            abd[t, :P, :P] = A[c * GPC + 2 * t]
            abd[t, P:, P:] = A[c * GPC + 2 * t + 1]
        cores.append({
            "x1": _pmaj(x1), "x2": _pmaj(x2),                    # [128, PAIRS, F]
            "x1t": x1t, "x2t": x2t,
            "abd": np.ascontiguousarray(abd.transpose(1, 0, 2)),  # [128, PAIRS, 128]
            "invbc": np.ascontiguousarray(np.broadcast_to(
                inv_deg[c * NPC:(c + 1) * NPC], (128, NPC))),
        })
    return cores


# ---------------------------------------------------------------- device kernel

def _build(nc):
    dt = nc.dram_tensor
    d = {}
    d["d_x1"] = dt("x1", [128, PAIRS, F], FP16, kind="ExternalInput")
    d["d_x2"] = dt("x2", [128, PAIRS, F], FP16, kind="ExternalInput")
    d["d_x1t"] = dt("x1t", [F, NPC], FP16, kind="ExternalInput")
    d["d_x2t"] = dt("x2t", [F, NPC], FP16, kind="ExternalInput")
    d["d_abd"] = dt("abd", [128, PAIRS, 128], FP16, kind="ExternalInput")
    d["d_invbc"] = dt("invbc", [128, NPC], F32, kind="ExternalInput")
    for li in range(4):
        ki = 1 if li == 0 else 2
        for nm in ("wl1", "wl2", "wr1", "wr2"):
            d[f"d_{nm}_{li}"] = dt(f"{nm}_{li}", [128, ki, H], FP16, kind="ExternalInput")
        d[f"d_b{li}"] = dt(f"b{li}", [128, 2], F32, kind="ExternalInput")
    d["d_w2"] = dt("w2", [128, 2 * KS, H], FP16, kind="ExternalInput")
    d["d_cb"] = dt("cb", [128, 2], F32, kind="ExternalInput")
    d["d_w1"] = dt("w1", [128, S1, H], FP16, kind="ExternalInput")
    d["d_lb1"] = dt("lb1", [GPC, H], F32, kind="ExternalInput")
    d["d_w4"] = dt("w4", [128, 2, 128], F32, kind="ExternalInput")
    d["d_b2q"] = dt("b2q", [128, 1], F32, kind="ExternalInput")
    d["d_w5"] = dt("w5", [128, N_CLASSES], F32, kind="ExternalInput")
    d["d_b3q"] = dt("b3q", [N_CLASSES, 1], F32, kind="ExternalInput")
    d["d_iota60"] = dt("iota60", [128, 2 * K], F32, kind="ExternalInput")
    d["d_off30"] = dt("off30", [128, 1], F32, kind="ExternalInput")
    d["d_epsrow"] = dt("epsrow", [P, P], F32, kind="ExternalInput")
    d["d_id128"] = dt("id128", [128, 128], F32, kind="ExternalInput")
    d["d_id16"] = dt("id16", [128, 128], FP16, kind="ExternalInput")
    d["d_out"] = dt("out", [GPC, N_CLASSES], F32, kind="ExternalOutput")

    with tile.TileContext(nc) as tc:
        _emit(tc, nc, d)
    nc.compile()
    return nc


def _ap(base, extra_offset, free_dims):
    """Custom AP view: replace partition+free dims relative to a tile AP."""
    return bass.AP(base.tensor, base.offset + extra_offset,
                   [base.ap[0]] + list(free_dims))


def _bap(base, extra_offset, dims):
    """AP with explicit partition dim (e.g. broadcast stride-0)."""
    return bass.AP(base.tensor, base.offset + extra_offset, list(dims))


def _emit(tc, nc, d):
    from contextlib import ExitStack
    ctx = ExitStack()
    with ctx:
        persist = ctx.enter_context(tc.tile_pool(name="persist", bufs=1))
        act_pool = ctx.enter_context(tc.tile_pool(name="acts", bufs=1))

        _deferred = []

        def load(name, shape, dram=None, dtype=F32):
            t = persist.tile(shape, dtype, tag=name)
            _deferred.append((t, (dram if dram is not None else d[f"d_{name}"]).ap()))
            return t

        wl1, wl2, wr1, wr2, bias = [], [], [], [], []
        for li in range(4):
            ki = 1 if li == 0 else 2
            for lst, nm in ((wl1, "wl1"), (wl2, "wl2"), (wr1, "wr1"), (wr2, "wr2")):
                lst.append(load(f"{nm}_{li}", [128, ki, H],
                                dram=d[f"d_{nm}_{li}"], dtype=FP16))
            bias.append(load(f"b{li}", [128, 2], dram=d[f"d_b{li}"]))
        id16 = load("id16", [128, 128], dtype=FP16)
        iota60 = load("iota60", [128, 2 * K])
        off30 = load("off30", [128, 1])
        epsrow = load("epsrow", [P, P])
        id128 = load("id128", [128, 128])
        w2 = load("w2", [128, 2 * KS, H], dtype=FP16)
        cb = load("cb", [128, 2])
        b1 = load("lb1", [GPC, H])
        w4 = load("w4", [128, 2, 128])
        b2q = load("b2q", [128, 1])
        w5 = load("w5", [128, N_CLASSES])
        b3q = load("b3q", [N_CLASSES, 1])
        inv_bc = persist.tile([128, NPC], F32, tag="invbc")
        w1sb = load("w1", [128, S1, H], dtype=FP16)

        # ---- activations that outlive the SAGE phase
        h1_sb = act_pool.tile([128, PAIRS, H], FP16, tag="h1")
        h2_sb = act_pool.tile([128, PAIRS, H], FP16, tag="h2")
        rt = act_pool.tile([P, P], F32, tag="rt")       # sort ranks, transposed

        with tc.tile_pool(name="sage", bufs=1) as sg:
            hts = [(sg.tile([128, 2, NPC], FP16, tag="hT1a", name="hT1a"),
                    sg.tile([128, 2, NPC], FP16, tag="hT2a", name="hT2a")),
                   (sg.tile([128, 2, NPC], FP16, tag="hT1b", name="hT1b"),
                    sg.tile([128, 2, NPC], FP16, tag="hT2b", name="hT2b"))]
            aggT1 = sg.tile([128, 2, NPC], FP16, tag="aggT1")
            aggT2 = sg.tile([128, 2, NPC], FP16, tag="aggT2")

            # ---- input DMAs, priority-ordered across 5 engine queues.
            # x pair lands straight in h1/h2_sb fh=0; xT pair in hts[0] slot 0
            # (they act as the "layer -1" activations).
            abd_sb = persist.tile([128, PAIRS, 128], FP16, tag="abd")
            first = [
                (_ap(h1_sb[:, :], 0, [[H, PAIRS], [1, F]]), d["d_x1"].ap()),
                (abd_sb[...], d["d_abd"].ap()),
                (_ap(h2_sb[:, :], 0, [[H, PAIRS], [1, F]]), d["d_x2"].ap()),
                (inv_bc[...], d["d_invbc"].ap()),
                (hts[0][0][:, 0, :], d["d_x1t"].ap()),
                (hts[0][1][:, 0, :], d["d_x2t"].ap()),
            ]
            queues = [nc.sync, nc.scalar, nc.gpsimd]
            for _i, (_dst, _src) in enumerate(first + _deferred):
                queues[_i % len(queues)].dma_start(
                    _dst if isinstance(_dst, bass.AP) else _dst[...], _src)
            _deferred.clear()

            with tc.tile_pool(name="ps_agg", bufs=2, space="PSUM") as psa, \
                 tc.tile_pool(name="ps_w", bufs=2, space="PSUM") as psw, \
                 tc.tile_pool(name="ps_tr", bufs=2, space="PSUM") as pst, \
                 tc.tile_pool(name="scr", bufs=2) as scr:
                for li in range(NLAYERS):
                    ki = 1 if li == 0 else 2
                    hT1v, hT2v = hts[li % 2]
                    hT1o, hT2o = hts[(li + 1) % 2]

                    # ---- agg: aggT[f, d] = (sum_s h[s,f] A[s,d]) * invdeg[d]
                    for g4 in range(PAIRS // 4):
                        for fh in range(ki):
                            ps = psa.tile([128, 4, 128], F32, tag="psa")
                            for j in range(4):
                                t = g4 * 4 + j
                                sl = slice(fh * 128, (fh + 1) * 128)
                                lh1, lh2 = h1_sb[:, t, sl], h2_sb[:, t, sl]
                                rhs = abd_sb[:, t, :]
                                nc.tensor.matmul(ps[:, j, :], lhsT=lh1, rhs=rhs,
                                                 start=True, stop=False)
                                nc.tensor.matmul(ps[:, j, :], lhsT=lh2, rhs=rhs,
                                                 start=False, stop=True)
                            tmp = scr.tile([128, 512], F32, tag="tmp")
                            nsl = slice(g4 * 512, (g4 + 1) * 512)
                            nc.vector.tensor_tensor(
                                tmp[...], _ap(ps[:, :], 0, [[1, 512]]),
                                inv_bc[:, nsl], op=mybir.AluOpType.mult)
                            nc.scalar.activation(
                                aggT1[:, fh, nsl], tmp[...],
                                mybir.ActivationFunctionType.Copy)
                            nc.vector.tensor_tensor(
                                aggT2[:, fh, nsl], tmp[...], aggT1[:, fh, nsl],
                                op=mybir.AluOpType.subtract)

                    # ---- weights: hT_next = relu(wl^T agg + wr^T h + b)
                    oh_order = (1, 0) if li == NLAYERS - 1 else (0, 1)
                    for oh in oh_order:
                        osl = slice(oh * 128, (oh + 1) * 128)
                        for ncki in range(NPC // NCHUNK):
                            sl = slice(ncki * NCHUNK, (ncki + 1) * NCHUNK)
                            ps = psw.tile([128, NCHUNK], F32, tag="psw")
                            prods = []
                            for m1, m2, r1, r2 in (
                                    (wl1[li], wl2[li], aggT1, aggT2),
                                    (wr1[li], wr2[li], hT1v, hT2v)):
                                for fh in range(ki):
                                    prods += [(m1, fh, r1), (m1, fh, r2),
                                              (m2, fh, r1)]
                            for i, (wm, fh, rt_) in enumerate(prods):
                                nc.tensor.matmul(
                                    ps[...], lhsT=wm[:, fh, osl],
                                    rhs=rt_[:, fh, sl],
                                    start=(i == 0), stop=(i == len(prods) - 1))
                            nc.scalar.activation(
                                hT1o[:, oh, sl], ps[...],
                                mybir.ActivationFunctionType.Relu,
                                bias=bias[li][:, oh:oh + 1])
                            if li < NLAYERS - 1 or oh == 1:
                                # (L3 needs hT2 only for the sort-key channel
                                # 255, i.e. the oh=1 half, row 127)
                                nc.vector.scalar_tensor_tensor(
                                    hT2o[:, oh, sl], ps[...], 0.0,
                                    hT1o[:, oh, sl],
                                    op0=mybir.AluOpType.max,
                                    op1=mybir.AluOpType.subtract)

                    # ---- transpose hT_next -> node-major h (fp16, batched 2 pairs)
                    parts = ((hT1o, h1_sb), (hT2o, h2_sb)) if li < NLAYERS - 1 \
                        else ((hT1o, h1_sb),)
                    for pi, (src_t, dst_t) in enumerate(parts):
                        for t0 in range(0, PAIRS, 2):
                            ps = pst.tile([128, 4, 128], FP16, tag="pst")
                            for dt_ in range(2):
                                for oh in range(2):
                                    nc.tensor.transpose(
                                        ps[:, dt_ * 2 + oh, :],
                                        src_t[:, oh, (t0 + dt_) * 128:(t0 + dt_ + 1) * 128],
                                        id16[...])
                            if (t0 // 2 + pi) % 2 == 0:
                                nc.scalar.activation(
                                    _ap(dst_t[:, :], t0 * H, [[1, 2 * H]]),
                                    _ap(ps[:, :], 0, [[1, 512]]),
                                    mybir.ActivationFunctionType.Copy)
                            else:
                                nc.vector.tensor_copy(
                                    _ap(dst_t[:, :], t0 * H, [[1, 2 * H]]),
                                    _ap(ps[:, :], 0, [[1, 512]]))

            # ---- sort keys: channel 255 = row 127 of (hT1+hT2)(L3)[:,1,:]
            with tc.tile_pool(name="sort", bufs=1) as ss:
                km1 = ss.tile([P, P], FP16, tag="km1")
                nc.sync.dma_start(km1[...], hts[NLAYERS % 2][0][127:128, 1, :])
                km2 = ss.tile([P, P], FP16, tag="km2")
                nc.scalar.dma_start(km2[...], hts[NLAYERS % 2][1][127:128, 1, :])
                kmf = ss.tile([P, P], F32, tag="kmf")
                nc.vector.tensor_add(kmf[...], km1[...], km2[...])
                kmp = ss.tile([P, P], F32, tag="kmp")
                nc.vector.tensor_sub(kmp[...], kmf[...], epsrow[...])
                cbt = ss.tile([P, P * P], mybir.dt.uint8, tag="cbt")
                kb = kmp[:, :]
                in0 = _ap(kb, 0, [[0, P], kb.ap[1]])       # [g, i(bc), j] k(g, j)
                in1 = _ap(kb, 0, [kb.ap[1], [0, P]])       # [g, i, j(bc)] k(g, i)
                nc.vector.tensor_tensor(
                    _ap(cbt[:, :], 0, [[P, P], [1, P]]), in0, in1,
                    op=mybir.AluOpType.is_gt)
                rk = ss.tile([P, P], F32, tag="rk")
                nc.vector.tensor_reduce(
                    rk[...], _ap(cbt[:, :], 0, [[P, P], [1, P]]),
                    axis=mybir.AxisListType.X, op=mybir.AluOpType.add)
                with tc.tile_pool(name="ps_sort", bufs=1, space="PSUM") as pss:
                    pr = pss.tile([P, P], F32, tag="pr")
                    nc.tensor.transpose(pr[...], rk[...], id128[0:P, 0:P])
                    nc.any.tensor_copy(rt[...], pr[...])

        # ---------------- selection + conv + mlp (sage pool closed)
        with tc.tile_pool(name="tail", bufs=1) as tp, \
             tc.tile_pool(name="ps_tail", bufs=2, space="PSUM") as ptl, \
             tc.tile_pool(name="ps_fin", bufs=1, space="PSUM") as pfin:
            # rankP[p, t] = rank(node p%64 of graph 2t + p//64)
            rankp = tp.tile([128, PAIRS], F32, tag="rankp")
            rb = rt[:, :]
            nc.vector.tensor_copy(rankp[0:P, :], _ap(rb, 0, [[2, PAIRS]]))
            nc.sync.dma_start(rankp[P:128, :], _ap(rb, 1, [[2, PAIRS]]))
            ge30 = tp.tile([128, PAIRS], F32, tag="ge30")
            nc.vector.tensor_scalar(ge30[...], rankp[...], float(K), None,
                                    op0=mybir.AluOpType.is_ge)
            rank2 = tp.tile([128, PAIRS], F32, tag="rank2")
            nc.vector.scalar_tensor_tensor(rank2[...], ge30[...], 1000.0,
                                           rankp[...], op0=mybir.AluOpType.mult,
                                           op1=mybir.AluOpType.add)
            nc.vector.tensor_scalar(rank2[...], rank2[...], off30[:, 0:1], None,
                                    op0=mybir.AluOpType.add)
            # one-hot selection matrices  PT[p, t, c] = (c == rank2[p, t])
            pt_all = tp.tile([128, PAIRS, 2 * K], FP16, tag="pt")
            io = iota60[:, :]
            r2 = rank2[:, :]
            nc.vector.tensor_tensor(
                pt_all[...],
                _ap(io, 0, [[0, PAIRS], [1, 2 * K]]),
                _ap(r2, 0, [[1, PAIRS], [0, 2 * K]]),
                op=mybir.AluOpType.is_equal)

            # topkT[f, b*30+r] = sum_n h1[n, f] * PT[n, b(pair), r]  (fp16)
            topkT = tp.tile([128, 2, TKPAD], FP16, tag="topkT")
            nc.vector.memset(topkT[:, :, GPC * K:].bitcast(F32), 0.0)
            for t0 in range(0, PAIRS, 4):
                ps = ptl.tile([128, 8, 2 * K], F32, tag="pssel")
                for j in range(4):
                    for mh in range(2):
                        nc.tensor.matmul(
                            ps[:, j * 2 + mh, :],
                            lhsT=h1_sb[:, t0 + j, mh * 128:(mh + 1) * 128],
                            rhs=pt_all[:, t0 + j, :],
                            start=True, stop=True)
                for mh in range(2):
                    nc.any.tensor_copy(
                        _ap(topkT[:, mh, :], t0 * 2 * K, [[1, 4 * 2 * K]]),
                        _bap(ps[:, :], mh * 2 * K, [ps.ap[0], [4 * K, 4], [1, 2 * K]]))

            # conv1d: y[p, oh, b, l] = relu(sum_{k, ih} w2^T topkT[:, b*30+l+k] + cb)
            y_sb = tp.tile([128, 2, GPC, L28], FP16, tag="y")
            for oh in range(2):
                for bc in range(GPC // GCHUNK):
                    ps = ptl.tile([128, GCHUNK, L28], F32, tag="psconv")
                    step = 0
                    for k in range(KS):
                        for ih in range(2):
                            base = topkT[:, ih, :]
                            rhs = _ap(base, bc * GCHUNK * K + k,
                                      [[K, GCHUNK], [1, L28]])
                            nc.tensor.matmul(
                                ps[...],
                                lhsT=w2[:, k * 2 + ih, oh * 128:(oh + 1) * 128],
                                rhs=rhs,
                                start=(step == 0), stop=(step == 2 * KS - 1))
                            step += 1
                    nc.scalar.activation(
                        y_sb[:, oh, bc * GCHUNK:(bc + 1) * GCHUNK, :], ps[...],
                        mybir.ActivationFunctionType.Relu,
                        bias=cb[:, oh:oh + 1])

            # lin1 (b-major): z1T[b, o] = relu(sum_s y_s^T @ w1_s + b1)
            ps1 = pfin.tile([GPC, H], F32, tag="ps1")
            for s in range(S1):
                ot, l = divmod(s, L_OUT)
                nc.tensor.matmul(
                    ps1[...],
                    lhsT=y_sb[:, ot, :, l],
                    rhs=w1sb[:, s, :],
                    start=(s == 0), stop=(s == S1 - 1))
            z1t = tp.tile([GPC, H], F32, tag="z1t")
            nc.vector.tensor_add(z1t[...], ps1[...], b1[...])
            nc.scalar.activation(z1t[...], z1t[...],
                                 mybir.ActivationFunctionType.Relu, bias=0.0)
            z1 = tp.tile([128, 2, GPC], F32, tag="z1")
            for mh in range(2):
                psz = pfin.tile([128, GPC], F32, tag="psz")
                nc.tensor.transpose(psz[...],
                                    z1t[:, mh * 128:(mh + 1) * 128],
                                    id128[0:GPC, 0:GPC])
                nc.any.tensor_copy(z1[:, mh, :], psz[...])

            # lin2 + out
            ps2 = pfin.tile([128, GPC], F32, tag="ps2")
            for kh in range(2):
                nc.tensor.matmul(ps2[...], lhsT=w4[:, kh, :], rhs=z1[:, kh, :],
                                 start=(kh == 0), stop=(kh == 1))
            z2 = tp.tile([128, GPC], F32, tag="z2")
            nc.scalar.activation(z2[...], ps2[...],
                                 mybir.ActivationFunctionType.Relu,
                                 bias=b2q[:, 0:1])
            ps3 = pfin.tile([N_CLASSES, GPC], F32, tag="ps3")
            nc.tensor.matmul(ps3[...], lhsT=w5[...], rhs=z2[...],
                             start=True, stop=True)
            o_sb = tp.tile([N_CLASSES, GPC], F32, tag="osb")
            nc.scalar.activation(o_sb[...], ps3[...],
                                 mybir.ActivationFunctionType.Relu,
                                 bias=b3q[:, 0:1])
            nc.sync.dma_start(d["d_out"].ap().rearrange("b o -> o b"), o_sb[...])


# ---------------------------------------------------------------- entry point

_CACHED = {}


def _get_nc():
    if "nc" not in _CACHED:
        nc = bacc.Bacc("TRN2", target_bir_lowering=False, debug=False,
                       enable_asserts=True)
        _CACHED["nc"] = _build(nc)
    return _CACHED["nc"]


def make_in_maps(inputs):
    sh = _prep_shared(inputs)
    cores = _prep_cores(inputs)
    return [{**sh, **c} for c in cores]


TRACE = False


def kernel(**inputs):
    in_maps = make_in_maps(inputs)
    nc = _get_nc()
    res = run_bass_kernel_spmd(nc, in_maps, core_ids=list(range(N_CORES)),
                               trace=TRACE)
    _CACHED["last_res"] = res
    return np.concatenate([r["out"] for r in res.results], axis=0)


if __name__ == "__main__":
    import reference
    inputs = {k: np.asarray(v) for k, v in reference.setup_inputs().items()}
    out = kernel(**inputs)
    print("out", out.shape, out.dtype)
